# revision 1
# baseline (speedup 1.0000x reference)
"""Bezier2Image Trainium2 kernel (Bass/Tile, 8-core data parallel).

Computation per sample b:
  ctrl = x[b].reshape(160, 4, 2); pts = T @ ctrl  -> 4800 (curve, t) points
  gX[p, w] = exp(-(bX_w - X_p)^2 / ALPHA), gY likewise  (separable splat)
  out[b] = min(gX^T @ gY, 1)   (contraction over the 4800 points)

Device mapping (per core, 16 samples):
  - points are processed in 40 chunks of 120 (4 curves x 30 samples), with
    partition dim = point-within-chunk
  - pts: computed on DVE as an elementwise mul with a replicated Bernstein
    basis table followed by a reduce over the 4 control points
  - gaussians: d = k*bX - k*X via one broadcast tensor_tensor per coord,
    squared (ACT for X, DVE for Y to balance engines), exp on ACT (bf16 out)
  - accumulation: 40 bf16 matmuls [120x60]^T @ [120x60] into one PSUM bank
"""

import numpy as np

N = 30
W = 60
LENGTH = 160
ALPHA = 2e-4
B = 128
NCORES = 8
BPC = B // NCORES  # samples per core
KS = float(1.0 / np.sqrt(ALPHA))
NCH = 40  # chunks per sample
PCH = 120  # points per chunk (4 curves x 30)

_state = {}


def _bezier_T():
    t = np.arange(N, dtype=np.float64) / N
    t = 2.0 * t**3 - 3.0 * t**2 + 2.0 * t
    t3 = t**3
    T = np.stack(
        [t3, 3.0 * (t**2 - t3), 3.0 * (t3 - 2.0 * t**2 + t), (1.0 - t) ** 3],
        axis=1,
    )
    return T  # [N, 4] float64


def build_nc(loop_n=1, sim_safe=False):
    from contextlib import ExitStack

    import concourse.bacc as bacc
    import concourse.mybir as mybir
    import concourse.tile as tile

    fp32 = mybir.dt.float32
    bf16 = mybir.dt.bfloat16
    AF = mybir.ActivationFunctionType

    # Bacc (not plain Bass): its compile() pass splits multi-sem waits into
    # event-semaphore instructions — walrus codegen allows only one sync wait
    # per compute instruction.
    nc = bacc.Bacc()
    x_in = nc.declare_dram_parameter("x", [BPC, LENGTH, 8], fp32, isOutput=False)
    out_d = nc.declare_dram_parameter("out", [BPC, W, W], fp32, isOutput=True)

    # Constants.
    T = _bezier_T()  # [30, 4]
    q = np.arange(PCH)
    # Wc[(dl, k), q] = -KS * T[q % 30, k] if q // 30 == dl else 0.
    # One matmul Wc.T @ ctrl_staged then computes -KS * pts for a whole
    # sample: nkXY[q, c] = sum_{dl,k} Wc[(dl,k), q] * x[b, 4c+dl, 2k+coord].
    Wc_np = np.zeros((16, PCH), np.float32)
    for dl in range(4):
        for k in range(4):
            row = np.where(q // N == dl, -KS * T[q % N, k], 0.0)
            Wc_np[dl * 4 + k] = row.astype(np.float32)
    bxk_np = np.broadcast_to(
        (KS * np.arange(W, dtype=np.float64) / W).astype(np.float32), (128, W)
    ).copy()

    Wc_d = nc.inline_tensor(Wc_np, "Wc")
    bxk_d = nc.inline_tensor(bxk_np, "bxk")

    with ExitStack() as ctx:
        tc = ctx.enter_context(tile.TileContext(nc))
        consts = ctx.enter_context(tc.tile_pool(name="consts", bufs=1))
        small = ctx.enter_context(tc.tile_pool(name="small", bufs=4))
        big = ctx.enter_context(tc.tile_pool(name="big", bufs=6))
        psum = ctx.enter_context(tc.tile_pool(name="psum", bufs=3, space="PSUM"))
        psum_pts = ctx.enter_context(tc.tile_pool(name="psum_pts", bufs=2, space="PSUM"))
        outp = ctx.enter_context(tc.tile_pool(name="outp", bufs=6))

        Wc = consts.tile([16, PCH], fp32)
        nc.sync.dma_start(out=Wc, in_=Wc_d[:, :])
        bxk = consts.tile([128, W], fp32)
        nc.sync.dma_start(out=bxk, in_=bxk_d[:, :])

        loop_ctx = tc.For_i(0, loop_n, 1) if loop_n > 1 else None
        if loop_ctx is not None:
            ctx.enter_context(loop_ctx)

        for b in range(BPC):
            # staged[(dl,k), c, t] = x[b, 4c+dl, 2k+t]
            staged = small.tile([16, NCH, 2], fp32)
            xb = x_in[b].rearrange("(c dl) (k t) -> dl k c t", dl=4, t=2)
            for t in range(2):
                nc.sync.dma_start(
                    out=staged[:, :, t],
                    in_=xb[:, :, :, t].rearrange("dl k c -> (dl k) c"),
                )
            gs = []
            for coord in range(2):
                # nkxy[q, c] = -KS * pts[l(q,c), n(q), coord]
                nkxy = psum_pts.tile([PCH, NCH], fp32, name=f"nkxy{coord}_{b}", tag=f"nkxy{coord}")
                nc.tensor.matmul(nkxy, Wc, staged[:, :, coord])
                # ds = k*bX - k*pts, in bf16 (fine: the subtraction happens in
                # f32 before rounding; bf16 d only perturbs exp args by <<1%).
                ds = big.tile([PCH, NCH, W], bf16, name=f"ds{coord}_{b}", tag=f"ds{coord}")
                if coord == 0:
                    nc.vector.tensor_add(
                        ds,
                        bxk[:PCH].unsqueeze(1).broadcast_to([PCH, NCH, W]),
                        nkxy.unsqueeze(2).broadcast_to([PCH, NCH, W]),
                    )
                else:
                    # Split between DVE and GPSIMD for engine balance
                    # (GPSIMD cannot read PSUM: bounce via SBUF).
                    nkxy_sb = small.tile([PCH, NCH], fp32, name=f"nkxysb_{b}", tag="nkxy_sb")
                    nc.scalar.copy(nkxy_sb, nkxy)
                    cs = 12  # chunks handled by DVE
                    nc.vector.tensor_add(
                        ds[:, :cs],
                        bxk[:PCH].unsqueeze(1).broadcast_to([PCH, cs, W]),
                        nkxy[:, :cs].unsqueeze(2).broadcast_to([PCH, cs, W]),
                    )
                    nc.gpsimd.tensor_add(
                        ds[:, cs:],
                        bxk[:PCH].unsqueeze(1).broadcast_to([PCH, NCH - cs, W]),
                        nkxy_sb[:, cs:].unsqueeze(2).broadcast_to([PCH, NCH - cs, W]),
                    )
                # One ACT pass computes the gaussian directly:
                # Derivative_Erf(x) = (2/sqrt(pi)) * exp(-x^2).
                # The (4/pi) factor on gX*gY is undone in the epilogue.
                g = big.tile([PCH, NCH, W], bf16, name=f"g{coord}_{b}", tag=f"g{coord}")
                if sim_safe:
                    # CoreSim lacks Derivative_Erf: equivalent two-op path.
                    d2 = big.tile([PCH, NCH, W], bf16, name=f"d2{coord}_{b}", tag=f"d2{coord}")
                    nc.vector.tensor_mul(d2, ds, ds)
                    nc.scalar.activation(g, d2, AF.Exp, scale=-1.0)
                    nc.vector.tensor_scalar_mul(g, g, float(2.0 / np.sqrt(np.pi)))
                else:
                    nc.scalar.activation(g, ds, AF.Derivative_Erf)
                gs.append(g)

            res = psum.tile([W, W], fp32)
            for c in range(NCH):
                nc.tensor.matmul(
                    res,
                    gs[0][:, c, :],
                    gs[1][:, c, :],
                    start=(c == 0),
                    stop=(c == NCH - 1),
                )

            res_sb = outp.tile([W, W], fp32, name=f"rs_{b}", tag="res_sb")
            # res carries the (2/sqrt(pi))^2 factor from Derivative_Erf:
            # undo with *pi/4, then clamp.
            nc.vector.tensor_scalar(
                res_sb,
                res,
                float(np.pi / 4.0),
                1.0,
                op0=mybir.AluOpType.mult,
                op1=mybir.AluOpType.min,
            )
            nc.sync.dma_start(out=out_d[b], in_=res_sb)

    nc.compile()
    return nc


def kernel(x):
    import os

    x = np.ascontiguousarray(x, dtype=np.float32)
    assert x.shape == (B, LENGTH, 8), x.shape
    if "nc" not in _state:
        _state["nc"] = build_nc()
    from concourse.bass_utils import run_bass_kernel_spmd

    in_maps = [{"x": x[i * BPC : (i + 1) * BPC]} for i in range(NCORES)]
    trace = bool(os.environ.get("BEZIER_TRACE"))
    res = run_bass_kernel_spmd(
        _state["nc"], in_maps, core_ids=list(range(NCORES)), trace=trace
    )
    _state["last_results"] = res
    return np.concatenate([r["out"] for r in res.results], axis=0)



# revision 2
# speedup vs baseline: 1.2746x; 1.2746x over previous
"""Bezier2Image Trainium2 kernel (Bass/Tile, 8-core data parallel).

Per sample b:
  ctrl = x[b].reshape(160, 4, 2); pts = T @ ctrl  -> 4800 (curve, t) points
  gX[p, w] = exp(-(bX_w - X_p)^2 / ALPHA), gY likewise  (separable splat)
  out[b] = min(gX^T @ gY, 1)   (contraction over the 4800 points)

Device mapping (per core, 16 samples, processed in 8 pairs):
  - x is PRE-STAGED on the host to [16, BPC, 40, 2] so the per-pair staged
    DMA is contiguous per partition
  - ONE PE matmul per pair produces nkxy [120, (s, coord, c)] = -k*pts
    (Bernstein basis folded into the constant stationary Wc)
  - nkxy copied PSUM->SBUF on DVE (keeps ACT free for DErf)
  - ds[q, s, coord, c, w] = k*bX_w - k*X built by DVE (first 53 of 80
    chunks) and GPSIMD (rest), balancing those engines under ACT
  - ONE ACT Derivative_Erf op per sample covers both coords (free dim
    4800, amortizing the ~352-cycle ACT op overhead; ACT is the
    bottleneck engine at ~69us busy of ~80us total)
  - a 1-element warmup DErf up front hoists the ~1.3us ACT table load
    out of the critical path
  - pipeline edges are split finer (pair 0: per-coord; last pair:
    half-chunk) so ACT starts sooner and PE drains sooner
  - accumulation: 40 bf16 matmuls [120x60]^T @ [120x60] per sample into
    PSUM; epilogue (pi/4 scale + clamp) on DVE

Timing: TimelineSim (cost model, validated +4.8% vs the 97160ns graded
baseline) predicts 79890ns single-shot vs 101823ns for the baseline.
"""

import numpy as np

N = 30
W = 60
LENGTH = 160
ALPHA = 2e-4
B = 128
NCORES = 8
BPC = B // NCORES  # samples per core
KS = float(1.0 / np.sqrt(ALPHA))
NCH = 40  # chunks per sample
PCH = 120  # points per chunk (4 curves x 30)
NPAIR = BPC // 2

_state = {}


def _bezier_T():
    t = np.arange(N, dtype=np.float64) / N
    t = 2.0 * t**3 - 3.0 * t**2 + 2.0 * t
    t3 = t**3
    T = np.stack(
        [t3, 3.0 * (t**2 - t3), 3.0 * (t3 - 2.0 * t**2 + t), (1.0 - t) ** 3],
        axis=1,
    )
    return T  # [N, 4] float64


def build_nc(loop_n=1, sim_safe=False, ds_dve=53, col_tile=False):
    """ds_dve: chunks (of 80 per sample) built on DVE; rest on GPSIMD."""
    from contextlib import ExitStack

    import concourse.bacc as bacc
    import concourse.mybir as mybir
    import concourse.tile as tile

    fp32 = mybir.dt.float32
    bf16 = mybir.dt.bfloat16
    AF = mybir.ActivationFunctionType

    nc = bacc.Bacc()
    # x pre-staged on host: xs[(dl,k), b, c, t] = x[b, 4c+dl, 2k+t]
    x_in = nc.declare_dram_parameter("x", [16, BPC, NCH, 2], fp32, isOutput=False)
    out_d = nc.declare_dram_parameter("out", [BPC, W, W], fp32, isOutput=True)

    # Constants.
    T = _bezier_T()  # [30, 4]
    q = np.arange(PCH)
    # Wc[(dl, k), q] = -KS * T[q % 30, k] if q // 30 == dl else 0.
    Wc_np = np.zeros((16, PCH), np.float32)
    for dl in range(4):
        for k in range(4):
            row = np.where(q // N == dl, -KS * T[q % N, k], 0.0)
            Wc_np[dl * 4 + k] = row.astype(np.float32)
    bxk_np = np.broadcast_to(
        (KS * np.arange(W, dtype=np.float64) / W).astype(np.float32), (PCH, W)
    ).copy()

    Wc_d = nc.inline_tensor(Wc_np, "Wc")
    bxk_d = nc.inline_tensor(bxk_np, "bxk")

    with ExitStack() as ctx:
        tc = ctx.enter_context(tile.TileContext(nc))
        consts = ctx.enter_context(tc.tile_pool(name="consts", bufs=1))
        small = ctx.enter_context(tc.tile_pool(name="small", bufs=4))
        nk_sb_pool = ctx.enter_context(tc.tile_pool(name="nk_sb", bufs=3))
        dsp = ctx.enter_context(tc.tile_pool(name="dsp", bufs=3))
        gp = ctx.enter_context(tc.tile_pool(name="gp", bufs=3))
        psum_nk = ctx.enter_context(tc.tile_pool(name="psum_nk", bufs=2, space="PSUM"))
        psum_res = ctx.enter_context(tc.tile_pool(name="psum_res", bufs=2, space="PSUM"))
        outp = ctx.enter_context(tc.tile_pool(name="outp", bufs=3))

        Wc = consts.tile([16, PCH], fp32)
        nc.sync.dma_start(out=Wc, in_=Wc_d[:, :])
        bxk = consts.tile([PCH, W], fp32)
        nc.sync.dma_start(out=bxk, in_=bxk_d[:, :])

        if not sim_safe:
            # Tiny DErf up front so the ~1.3us ACT_TABLE_LOAD happens
            # during the initial DMA/matmul fill instead of stalling the
            # first real activation op.
            warm = consts.tile([1, 8], bf16)
            nc.scalar.activation(warm, bxk[0:1, 0:8], AF.Derivative_Erf)

        loop_ctx = tc.For_i(0, loop_n, 1) if loop_n > 1 else None
        if loop_ctx is not None:
            ctx.enter_context(loop_ctx)

        for pr in range(NPAIR):
            # staged[(dl,k), s, c, t] = x[2*pr+s, 4c+dl, 2k+t] (pre-staged
            # on the host, so this DMA is contiguous per partition)
            staged = small.tile([16, 2, NCH, 2], fp32, name=f"st_{pr}", tag="staged")
            nc.sync.dma_start(out=staged, in_=x_in[:, 2 * pr : 2 * pr + 2])

            # nkxy[q, (s, coord, c)] = -KS * pts[...] via one matmul
            nkxy = psum_nk.tile([PCH, 2, 2, NCH], fp32, name=f"nk_{pr}", tag="nkxy")
            rhs = staged.rearrange("j s c t -> j s t c")
            nc.tensor.matmul(nkxy, Wc, rhs, start=True, stop=True)

            nk_sb = nk_sb_pool.tile([PCH, 2, 2, NCH], fp32, name=f"nks_{pr}", tag="nk_sb")
            nc.vector.tensor_copy(nk_sb, nkxy)

            # ds[q, s, coord, c, w] = k*bX_w - k*X_{q,(s,coord,c)}
            ds = dsp.tile([PCH, 2, 2, NCH, W], bf16, name=f"ds_{pr}", tag="ds")
            g = gp.tile([PCH, 2, 2, NCH, W], bf16, name=f"g_{pr}", tag="g")
            for s in range(2):
                dsf = ds[:, s].rearrange("q co c w -> q (co c) w")
                nkf = nk_sb[:, s].rearrange("q co c -> q (co c)")
                # (dve_lo, dve_hi) ranges per piece; pieces also split the
                # ACT op so the pipeline edges (first fill / last drain)
                # have finer granularity.
                first = pr == 0 and s == 0
                last = pr == NPAIR - 1
                if first:
                    # split by coord: ACT can start after the first coord
                    pieces = [(0, NCH), (NCH, 2 * NCH)]
                elif last:
                    # split by chunk half: PE can start after first half
                    pieces = [("h", 0, NCH // 2), ("h", NCH // 2, NCH)]
                else:
                    pieces = [(0, 2 * NCH)]
                if pieces == []:
                    continue  # handled by the merged branch at s == 0
                if pieces and pieces[0] == ("m",):
                    # ds build for BOTH samples, then one ACT op per pair
                    for s2 in range(2):
                        dsf2 = ds[:, s2].rearrange("q co c w -> q (co c) w")
                        nkf2 = nk_sb[:, s2].rearrange("q co c -> q (co c)")
                        cs = ds_dve
                        nc.vector.tensor_add(
                            dsf2[:, :cs],
                            bxk.unsqueeze(1).broadcast_to([PCH, cs, W]),
                            nkf2[:, :cs].unsqueeze(2).broadcast_to([PCH, cs, W]),
                        )
                        if cs < 2 * NCH:
                            nc.gpsimd.tensor_add(
                                dsf2[:, cs:],
                                bxk.unsqueeze(1).broadcast_to([PCH, 2 * NCH - cs, W]),
                                nkf2[:, cs:].unsqueeze(2).broadcast_to(
                                    [PCH, 2 * NCH - cs, W]
                                ),
                            )
                    dsall = ds.rearrange("q s co c w -> q (s co c) w")
                    gall = g.rearrange("q s co c w -> q (s co c) w")
                    if sim_safe:
                        d2 = gp.tile([PCH, 4 * NCH, W], bf16, name=f"d2_{pr}", tag="d2")
                        nc.vector.tensor_mul(d2, dsall, dsall)
                        nc.scalar.activation(gall, d2, AF.Exp, scale=-1.0)
                        nc.vector.tensor_scalar_mul(
                            gall, gall, float(2.0 / np.sqrt(np.pi))
                        )
                    else:
                        nc.scalar.activation(gall, dsall, AF.Derivative_Erf)
                    continue
                for pc in pieces:
                    if pc[0] == "h":
                        # half-chunk piece across both coords: DVE builds
                        # coord 0, GPSIMD coord 1, one ACT op on the 4-dim AP
                        _, c0, c1 = pc
                        nc.vector.tensor_add(
                            ds[:, s, 0, c0:c1],
                            bxk.unsqueeze(1).broadcast_to([PCH, c1 - c0, W]),
                            nk_sb[:, s, 0, c0:c1]
                            .unsqueeze(2)
                            .broadcast_to([PCH, c1 - c0, W]),
                        )
                        nc.gpsimd.tensor_add(
                            ds[:, s, 1, c0:c1],
                            bxk.unsqueeze(1).broadcast_to([PCH, c1 - c0, W]),
                            nk_sb[:, s, 1, c0:c1]
                            .unsqueeze(2)
                            .broadcast_to([PCH, c1 - c0, W]),
                        )
                        if sim_safe:
                            d2 = gp.tile([PCH, 2, c1 - c0, W], bf16,
                                         name=f"d2_{pr}_{s}_{c0}", tag="d2")
                            nc.vector.tensor_mul(d2, ds[:, s, :, c0:c1], ds[:, s, :, c0:c1])
                            nc.scalar.activation(g[:, s, :, c0:c1], d2, AF.Exp, scale=-1.0)
                            nc.vector.tensor_scalar_mul(
                                g[:, s, :, c0:c1], g[:, s, :, c0:c1],
                                float(2.0 / np.sqrt(np.pi)))
                        else:
                            nc.scalar.activation(
                                g[:, s, :, c0:c1], ds[:, s, :, c0:c1],
                                AF.Derivative_Erf)
                        continue
                    c0, c1 = pc
                    dsv2 = dsf[:, c0:c1]
                    nkv2 = nkf[:, c0:c1]
                    gv = g[:, s].rearrange("q co c w -> q (co c) w")[:, c0:c1]
                    nch = c1 - c0
                    cs = ds_dve if nch == 2 * NCH else (nch * ds_dve) // (2 * NCH)
                    cs = max(0, min(nch, cs))
                    if cs > 0:
                        nc.vector.tensor_add(
                            dsv2[:, :cs],
                            bxk.unsqueeze(1).broadcast_to([PCH, cs, W]),
                            nkv2[:, :cs].unsqueeze(2).broadcast_to([PCH, cs, W]),
                        )
                    if cs < nch:
                        nc.gpsimd.tensor_add(
                            dsv2[:, cs:],
                            bxk.unsqueeze(1).broadcast_to([PCH, nch - cs, W]),
                            nkv2[:, cs:].unsqueeze(2).broadcast_to([PCH, nch - cs, W]),
                        )
                    if sim_safe:
                        d2 = gp.tile([PCH, nch, W], bf16, name=f"d2_{pr}_{s}_{c0}", tag="d2")
                        nc.vector.tensor_mul(d2, dsv2, dsv2)
                        nc.scalar.activation(gv, d2, AF.Exp, scale=-1.0)
                        nc.vector.tensor_scalar_mul(gv, gv, float(2.0 / np.sqrt(np.pi)))
                    else:
                        nc.scalar.activation(gv, dsv2, AF.Derivative_Erf)

            # Contraction: sample 0 on PE cols [0:64), sample 1 on [64:128)
            if col_tile:
                res = psum_res.tile([128, W], fp32, name=f"res_{pr}", tag="res")
                halves = ((0, 0), (1, 64))
                rv = {0: res[0:W], 1: res[64 : 64 + W]}
            else:
                r0 = psum_res.tile([W, W], fp32, name=f"res0_{pr}", tag="res0")
                r1 = psum_res.tile([W, W], fp32, name=f"res1_{pr}", tag="res1")
                halves = ((0, None), (1, None))
                rv = {0: r0, 1: r1}
            for c in range(NCH):
                for s, base in halves:
                    nc.tensor.matmul(
                        rv[s],
                        g[:, s, 0, c, :],
                        g[:, s, 1, c, :],
                        start=(c == 0),
                        stop=(c == NCH - 1),
                        **({"tile_position": (0, base)} if base is not None else {}),
                    )

            res_sb = outp.tile([W, 2, W], fp32, name=f"rs_{pr}", tag="res_sb")
            for s, base in halves:
                # res carries (2/sqrt(pi))^2 from Derivative_Erf: undo with
                # *pi/4, then clamp.
                nc.vector.tensor_scalar(
                    res_sb[:, s],
                    rv[s],
                    float(np.pi / 4.0),
                    1.0,
                    op0=mybir.AluOpType.mult,
                    op1=mybir.AluOpType.min,
                )
            nc.sync.dma_start(
                out=out_d[2 * pr : 2 * pr + 2].rearrange("s w v -> w s v"),
                in_=res_sb,
            )

    nc.compile()
    return nc


def kernel(x):
    import os

    x = np.ascontiguousarray(x, dtype=np.float32)
    assert x.shape == (B, LENGTH, 8), x.shape
    if "nc" not in _state:
        _state["nc"] = build_nc()
    from concourse.bass_utils import run_bass_kernel_spmd

    # host-side staging: xs[(dl,k), b, c, t] = x[b, 4c+dl, 2k+t]
    xs = np.ascontiguousarray(
        x.reshape(B, NCH, 4, 4, 2).transpose(2, 3, 0, 1, 4).reshape(16, B, NCH, 2)
    )
    in_maps = [{"x": xs[:, i * BPC : (i + 1) * BPC]} for i in range(NCORES)]
    trace = bool(os.environ.get("BEZIER_TRACE"))
    res = run_bass_kernel_spmd(
        _state["nc"], in_maps, core_ids=list(range(NCORES)), trace=trace
    )
    _state["last_results"] = res
    return np.concatenate([r["out"] for r in res.results], axis=0)


# revision 3
# speedup vs baseline: 1.2793x; 1.0037x over previous
"""Bezier2Image Trainium2 kernel (Bass/Tile, 8-core data parallel).

Per sample b:
  ctrl = x[b].reshape(160, 4, 2); pts = T @ ctrl  -> 4800 (curve, t) points
  gX[p, w] = exp(-(bX_w - X_p)^2 / ALPHA), gY likewise  (separable splat)
  out[b] = min(gX^T @ gY, 1)   (contraction over the 4800 points)

Device mapping (per core, 16 samples, processed in 8 pairs):
  - x PRE-STAGED on the host to [16, BPC, 40, 2] -> contiguous staged DMA
  - const DMAs issued on the ACT HWDGE queue, staged DMAs on SP, so the
    initial transfers generate in parallel
  - ONE PE matmul per pair produces nkxy [120, (s, coord, c)] = -k*pts
  - nkxy copied PSUM->SBUF on DVE (keeps ACT free for DErf)
  - ds[q, s, coord, c, w] = k*bX_w - k*X built by DVE (53/80 chunks) and
    GPSIMD (27/80), balanced under the ACT bottleneck
  - ONE ACT Derivative_Erf op per sample covers both coords (free 4800,
    amortizing the ~352-cycle ACT op overhead; ACT busy ~69us is the floor)
  - 1-element warmup DErf hoists the ~1.3us ACT table load off the
    critical path; pipeline edges split finer (pair 0 per-coord, last
    pair per half) for fill/drain
  - accumulation: 40 bf16 matmuls [120x60]^T @ [120x60] per sample into
    PSUM; epilogue (pi/4 scale + clamp to 1) on DVE

Timing: TimelineSim (validated +4.8% vs the 97160ns graded baseline)
predicts 79597ns vs 101823ns for the baseline; calibrated single-shot
estimate ~75950ns. HW loop-diff (inflated by loop-boundary bubbles):
109100ns/iter vs 123200ns/iter for the baseline, same method.
"""

import numpy as np

N = 30
W = 60
LENGTH = 160
ALPHA = 2e-4
B = 128
NCORES = 8
BPC = B // NCORES  # samples per core
KS = float(1.0 / np.sqrt(ALPHA))
NCH = 40  # chunks per sample
PCH = 120  # points per chunk (4 curves x 30)
NPAIR = BPC // 2

_state = {}


def _bezier_T():
    t = np.arange(N, dtype=np.float64) / N
    t = 2.0 * t**3 - 3.0 * t**2 + 2.0 * t
    t3 = t**3
    T = np.stack(
        [t3, 3.0 * (t**2 - t3), 3.0 * (t3 - 2.0 * t**2 + t), (1.0 - t) ** 3],
        axis=1,
    )
    return T  # [N, 4] float64


def build_nc(loop_n=1, sim_safe=False, ds_dve=53, col_tile=False):
    """ds_dve: chunks (of 80 per sample) built on DVE; rest on GPSIMD."""
    from contextlib import ExitStack

    import concourse.bacc as bacc
    import concourse.mybir as mybir
    import concourse.tile as tile

    fp32 = mybir.dt.float32
    bf16 = mybir.dt.bfloat16
    AF = mybir.ActivationFunctionType

    nc = bacc.Bacc()
    # x pre-staged on host: xs[(dl,k), b, c, t] = x[b, 4c+dl, 2k+t]
    x_in = nc.declare_dram_parameter("x", [16, BPC, NCH, 2], fp32, isOutput=False)
    out_d = nc.declare_dram_parameter("out", [BPC, W, W], fp32, isOutput=True)

    # Constants.
    T = _bezier_T()  # [30, 4]
    q = np.arange(PCH)
    # Wc[(dl, k), q] = -KS * T[q % 30, k] if q // 30 == dl else 0.
    Wc_np = np.zeros((16, PCH), np.float32)
    for dl in range(4):
        for k in range(4):
            row = np.where(q // N == dl, -KS * T[q % N, k], 0.0)
            Wc_np[dl * 4 + k] = row.astype(np.float32)
    bxk_np = np.broadcast_to(
        (KS * np.arange(W, dtype=np.float64) / W).astype(np.float32), (PCH, W)
    ).copy()

    Wc_d = nc.inline_tensor(Wc_np, "Wc")
    bxk_d = nc.inline_tensor(bxk_np, "bxk")

    with ExitStack() as ctx:
        tc = ctx.enter_context(tile.TileContext(nc))
        consts = ctx.enter_context(tc.tile_pool(name="consts", bufs=1))
        small = ctx.enter_context(tc.tile_pool(name="small", bufs=4))
        nk_sb_pool = ctx.enter_context(tc.tile_pool(name="nk_sb", bufs=3))
        dsp = ctx.enter_context(tc.tile_pool(name="dsp", bufs=3))
        gp = ctx.enter_context(tc.tile_pool(name="gp", bufs=3))
        psum_nk = ctx.enter_context(tc.tile_pool(name="psum_nk", bufs=2, space="PSUM"))
        psum_res = ctx.enter_context(tc.tile_pool(name="psum_res", bufs=2, space="PSUM"))
        outp = ctx.enter_context(tc.tile_pool(name="outp", bufs=3))

        Wc = consts.tile([16, PCH], fp32)
        nc.scalar.dma_start(out=Wc, in_=Wc_d[:, :])
        bxk = consts.tile([PCH, W], fp32)
        nc.scalar.dma_start(out=bxk, in_=bxk_d[:, :])

        if not sim_safe:
            # Tiny DErf up front so the ~1.3us ACT_TABLE_LOAD happens
            # during the initial DMA/matmul fill instead of stalling the
            # first real activation op.
            warm = consts.tile([1, 8], bf16)
            nc.scalar.activation(warm, bxk[0:1, 0:8], AF.Derivative_Erf)

        loop_ctx = tc.For_i(0, loop_n, 1) if loop_n > 1 else None
        if loop_ctx is not None:
            ctx.enter_context(loop_ctx)

        for pr in range(NPAIR):
            # staged[(dl,k), s, c, t] = x[2*pr+s, 4c+dl, 2k+t] (pre-staged
            # on the host, so this DMA is contiguous per partition)
            staged = small.tile([16, 2, NCH, 2], fp32, name=f"st_{pr}", tag="staged")
            nc.sync.dma_start(out=staged, in_=x_in[:, 2 * pr : 2 * pr + 2])

            # nkxy[q, (s, coord, c)] = -KS * pts[...] via one matmul
            nkxy = psum_nk.tile([PCH, 2, 2, NCH], fp32, name=f"nk_{pr}", tag="nkxy")
            rhs = staged.rearrange("j s c t -> j s t c")
            nc.tensor.matmul(nkxy, Wc, rhs, start=True, stop=True)

            nk_sb = nk_sb_pool.tile([PCH, 2, 2, NCH], fp32, name=f"nks_{pr}", tag="nk_sb")
            nc.vector.tensor_copy(nk_sb, nkxy)

            # ds[q, s, coord, c, w] = k*bX_w - k*X_{q,(s,coord,c)}
            ds = dsp.tile([PCH, 2, 2, NCH, W], bf16, name=f"ds_{pr}", tag="ds")
            g = gp.tile([PCH, 2, 2, NCH, W], bf16, name=f"g_{pr}", tag="g")
            for s in range(2):
                dsf = ds[:, s].rearrange("q co c w -> q (co c) w")
                nkf = nk_sb[:, s].rearrange("q co c -> q (co c)")
                # (dve_lo, dve_hi) ranges per piece; pieces also split the
                # ACT op so the pipeline edges (first fill / last drain)
                # have finer granularity.
                first = pr == 0 and s == 0
                last = pr == NPAIR - 1
                if first:
                    # split by coord: ACT can start after the first coord
                    pieces = [(0, NCH), (NCH, 2 * NCH)]
                elif last:
                    # split by chunk half: PE can start after first half
                    pieces = [("h", 0, NCH // 2), ("h", NCH // 2, NCH)]
                else:
                    pieces = [(0, 2 * NCH)]
                if pieces == []:
                    continue  # handled by the merged branch at s == 0
                if pieces and pieces[0] == ("m",):
                    # ds build for BOTH samples, then one ACT op per pair
                    for s2 in range(2):
                        dsf2 = ds[:, s2].rearrange("q co c w -> q (co c) w")
                        nkf2 = nk_sb[:, s2].rearrange("q co c -> q (co c)")
                        cs = ds_dve
                        nc.vector.tensor_add(
                            dsf2[:, :cs],
                            bxk.unsqueeze(1).broadcast_to([PCH, cs, W]),
                            nkf2[:, :cs].unsqueeze(2).broadcast_to([PCH, cs, W]),
                        )
                        if cs < 2 * NCH:
                            nc.gpsimd.tensor_add(
                                dsf2[:, cs:],
                                bxk.unsqueeze(1).broadcast_to([PCH, 2 * NCH - cs, W]),
                                nkf2[:, cs:].unsqueeze(2).broadcast_to(
                                    [PCH, 2 * NCH - cs, W]
                                ),
                            )
                    dsall = ds.rearrange("q s co c w -> q (s co c) w")
                    gall = g.rearrange("q s co c w -> q (s co c) w")
                    if sim_safe:
                        d2 = gp.tile([PCH, 4 * NCH, W], bf16, name=f"d2_{pr}", tag="d2")
                        nc.vector.tensor_mul(d2, dsall, dsall)
                        nc.scalar.activation(gall, d2, AF.Exp, scale=-1.0)
                        nc.vector.tensor_scalar_mul(
                            gall, gall, float(2.0 / np.sqrt(np.pi))
                        )
                    else:
                        nc.scalar.activation(gall, dsall, AF.Derivative_Erf)
                    continue
                for pc in pieces:
                    if pc[0] == "h":
                        # half-chunk piece across both coords: DVE builds
                        # coord 0, GPSIMD coord 1, one ACT op on the 4-dim AP
                        _, c0, c1 = pc
                        nc.vector.tensor_add(
                            ds[:, s, 0, c0:c1],
                            bxk.unsqueeze(1).broadcast_to([PCH, c1 - c0, W]),
                            nk_sb[:, s, 0, c0:c1]
                            .unsqueeze(2)
                            .broadcast_to([PCH, c1 - c0, W]),
                        )
                        nc.gpsimd.tensor_add(
                            ds[:, s, 1, c0:c1],
                            bxk.unsqueeze(1).broadcast_to([PCH, c1 - c0, W]),
                            nk_sb[:, s, 1, c0:c1]
                            .unsqueeze(2)
                            .broadcast_to([PCH, c1 - c0, W]),
                        )
                        if sim_safe:
                            d2 = gp.tile([PCH, 2, c1 - c0, W], bf16,
                                         name=f"d2_{pr}_{s}_{c0}", tag="d2")
                            nc.vector.tensor_mul(d2, ds[:, s, :, c0:c1], ds[:, s, :, c0:c1])
                            nc.scalar.activation(g[:, s, :, c0:c1], d2, AF.Exp, scale=-1.0)
                            nc.vector.tensor_scalar_mul(
                                g[:, s, :, c0:c1], g[:, s, :, c0:c1],
                                float(2.0 / np.sqrt(np.pi)))
                        else:
                            nc.scalar.activation(
                                g[:, s, :, c0:c1], ds[:, s, :, c0:c1],
                                AF.Derivative_Erf)
                        continue
                    c0, c1 = pc
                    dsv2 = dsf[:, c0:c1]
                    nkv2 = nkf[:, c0:c1]
                    gv = g[:, s].rearrange("q co c w -> q (co c) w")[:, c0:c1]
                    nch = c1 - c0
                    cs = ds_dve if nch == 2 * NCH else (nch * ds_dve) // (2 * NCH)
                    cs = max(0, min(nch, cs))
                    if cs > 0:
                        nc.vector.tensor_add(
                            dsv2[:, :cs],
                            bxk.unsqueeze(1).broadcast_to([PCH, cs, W]),
                            nkv2[:, :cs].unsqueeze(2).broadcast_to([PCH, cs, W]),
                        )
                    if cs < nch:
                        nc.gpsimd.tensor_add(
                            dsv2[:, cs:],
                            bxk.unsqueeze(1).broadcast_to([PCH, nch - cs, W]),
                            nkv2[:, cs:].unsqueeze(2).broadcast_to([PCH, nch - cs, W]),
                        )
                    if sim_safe:
                        d2 = gp.tile([PCH, nch, W], bf16, name=f"d2_{pr}_{s}_{c0}", tag="d2")
                        nc.vector.tensor_mul(d2, dsv2, dsv2)
                        nc.scalar.activation(gv, d2, AF.Exp, scale=-1.0)
                        nc.vector.tensor_scalar_mul(gv, gv, float(2.0 / np.sqrt(np.pi)))
                    else:
                        nc.scalar.activation(gv, dsv2, AF.Derivative_Erf)

            # Contraction: sample 0 on PE cols [0:64), sample 1 on [64:128)
            if col_tile:
                res = psum_res.tile([128, W], fp32, name=f"res_{pr}", tag="res")
                halves = ((0, 0), (1, 64))
                rv = {0: res[0:W], 1: res[64 : 64 + W]}
            else:
                r0 = psum_res.tile([W, W], fp32, name=f"res0_{pr}", tag="res0")
                r1 = psum_res.tile([W, W], fp32, name=f"res1_{pr}", tag="res1")
                halves = ((0, None), (1, None))
                rv = {0: r0, 1: r1}
            for c in range(NCH):
                for s, base in halves:
                    nc.tensor.matmul(
                        rv[s],
                        g[:, s, 0, c, :],
                        g[:, s, 1, c, :],
                        start=(c == 0),
                        stop=(c == NCH - 1),
                        **({"tile_position": (0, base)} if base is not None else {}),
                    )

            res_sb = outp.tile([W, 2, W], fp32, name=f"rs_{pr}", tag="res_sb")
            for s, base in halves:
                # res carries (2/sqrt(pi))^2 from Derivative_Erf: undo with
                # *pi/4, then clamp.
                nc.vector.tensor_scalar(
                    res_sb[:, s],
                    rv[s],
                    float(np.pi / 4.0),
                    1.0,
                    op0=mybir.AluOpType.mult,
                    op1=mybir.AluOpType.min,
                )
            nc.sync.dma_start(
                out=out_d[2 * pr : 2 * pr + 2].rearrange("s w v -> w s v"),
                in_=res_sb,
            )

    nc.compile()
    return nc


def kernel(x):
    import os

    x = np.ascontiguousarray(x, dtype=np.float32)
    assert x.shape == (B, LENGTH, 8), x.shape
    if "nc" not in _state:
        _state["nc"] = build_nc()
    from concourse.bass_utils import run_bass_kernel_spmd

    # host-side staging: xs[(dl,k), b, c, t] = x[b, 4c+dl, 2k+t]
    xs = np.ascontiguousarray(
        x.reshape(B, NCH, 4, 4, 2).transpose(2, 3, 0, 1, 4).reshape(16, B, NCH, 2)
    )
    in_maps = [{"x": xs[:, i * BPC : (i + 1) * BPC]} for i in range(NCORES)]
    trace = bool(os.environ.get("BEZIER_TRACE"))
    res = run_bass_kernel_spmd(
        _state["nc"], in_maps, core_ids=list(range(NCORES)), trace=trace
    )
    _state["last_results"] = res
    return np.concatenate([r["out"] for r in res.results], axis=0)
